# revision 1
# baseline (speedup 1.0000x reference)
"""GAT (3-layer, PyG-style) on 8 Trainium2 NeuronCores via Bass/Tile.

Sharding: nodes are split across 8 cores by destination-node range
(12500/core, padded to 12544 = 98*128).  Edges (incl. self-loops) are
partitioned by dst owner, sorted by dst, bucketed by src shard (4 shards
so src row ids fit dma_gather's int16 indices), and packed host-side
into fixed tiles of 128 edges.

Per layer, each core computes fused per-node rows
  hxp[n] = [h bf16 x64 | asrc f32 x8 | adst f32 x8 | pad]  (256 B)
for its own nodes with PE matmuls, AllGathers the table, then per
dst-block: dma_gathers edge source rows (by src, 4 shards) and dst rows
(by dst, local), builds one-hot edge->dst selection matrices with
is_equal against an iota constant, computes w = exp(leaky_relu(
asrc+adst)) with batched DVE/ACT ops, and aggregates num/den with PE
selection matmuls accumulating in PSUM.  Softmax normalization happens
after aggregation: out = (sum_j w_j h_j) / (sum_j w_j), which equals the
reference's max-stabilized softmax exactly (in exact arithmetic).
Layer 2 keeps per-head numerators (8 heads x 64 feats in PSUM) and
applies W2 / head-mean / log_softmax after normalization.
"""

import math
from dataclasses import dataclass

import numpy as np

# ----------------------------------------------------------------------------
# Problem constants (hardcoded per the harness contract)
# ----------------------------------------------------------------------------
N = 100000
F_IN = 512
H = 8
C_HID = 8
N_CLS = 40
E = 1600000
NEG_SLOPE = 0.2
M_CORES = 8
HID = H * C_HID  # 64
NSHARD = 4       # src shards so gather indices fit int16
ROWW = 128       # table row width in bf16 elements (256 B)
PHASES = "full"  # debug knob: "a" | "ag" | "b0" | "full"


@dataclass(frozen=True)
class Cfg:
    m: int          # cores
    n: int          # total nodes
    f_in: int       # input features (multiple of 128)
    ncls: int       # classes
    npc: int        # nodes per core
    b: int          # dst blocks per core (npc padded to b*128)
    tb: int         # edge tiles (of 128) per (block, src-shard) bucket
    sb: int         # blocks per gather superblock
    hid: int = 64   # H*C
    heads: int = 8

    @property
    def npad(self):
        return self.b * 128

    @property
    def fk(self):
        return self.f_in // 128  # K chunks for layer-0 matmul

    @property
    def t(self):
        return NSHARD * self.tb  # tiles per block

    @property
    def nsb(self):
        return self.b // self.sb  # superblocks per core

    @property
    def kpsb(self):
        return self.sb * self.t  # chunks per superblock

    @property
    def shard_rows(self):
        return self.m * self.npad // NSHARD


# ----------------------------------------------------------------------------
# Host-side preprocessing
# ----------------------------------------------------------------------------

def _block_diag_a(a):
    Hh, Cc = a.shape
    out = np.zeros((Hh * Cc, Hh), dtype=np.float32)
    for h in range(Hh):
        out[h * Cc:(h + 1) * Cc, h] = a[h]
    return out


def _wrap16(flat):
    """Pack a flat int16 index list into the [128, n/16] dma_gather layout
    (idx i at [i%16, i//16], replicated to all 8 Q7 core groups)."""
    n = flat.shape[-1]
    assert n % 16 == 0
    arr = flat.reshape(*flat.shape[:-1], n // 16, 16)
    arr = np.swapaxes(arr, -1, -2)          # [..., 16, n//16]
    reps = (1,) * (arr.ndim - 2) + (8, 1)
    return np.tile(arr, reps).astype(np.int16)


def _pack_edges(cfg: Cfg, src, dst, force_tb=None):
    """Returns per-core packed gather indices + dstc, plus tb."""
    m, npc, b, sbn = cfg.m, cfg.npc, cfg.b, cfg.sb
    owner = dst // npc
    shard_rows = cfg.m * cfg.npad // NSHARD
    per_core = []
    max_tb = 1
    for mm in range(m):
        sel = np.nonzero(owner == mm)[0]
        dl = (dst[sel] - mm * npc).astype(np.int64)
        sg = src[sel]
        sg = (sg // npc) * cfg.npad + (sg % npc)      # global padded row id
        blk = dl // 128
        shard = sg // shard_rows
        key = blk * NSHARD + shard
        order = np.argsort(key, kind="stable")
        dl, sg, key = dl[order], sg[order], key[order]
        counts = np.bincount(key, minlength=b * NSHARD)
        max_tb = max(max_tb, int(math.ceil(counts.max() / 128)))
        per_core.append((sg, dl, counts))

    tb = force_tb if force_tb is not None else max_tb
    t = NSHARD * tb
    nsb = b // sbn
    kpsb = sbn * t
    packed = []
    for mm in range(m):
        sg, dl, counts = per_core[mm]
        # g1 idx (shard-local, int16), per superblock per shard
        g1i = np.zeros((nsb, NSHARD, sbn * tb * 128), dtype=np.int16)
        g2i = np.zeros((nsb, kpsb * 128), dtype=np.int16)
        dstc = np.full((nsb, 128, kpsb), 30000.0, dtype=np.float16)
        pos = 0
        for bb in range(b):
            sbi, bi = bb // sbn, bb % sbn
            for s in range(NSHARD):
                cnt = int(counts[bb * NSHARD + s])
                ss = sg[pos:pos + cnt] - s * shard_rows
                dd = dl[pos:pos + cnt]
                pos += cnt
                i = np.arange(cnt)
                # position inside this shard's superblock gather list
                q = bi * (tb * 128) + i
                g1i[sbi, s, q] = ss.astype(np.int16)
                # canonical chunk index within superblock
                kk = s * (sbn * tb) + bi * tb + i // 128
                pp = i % 128
                g2i[sbi, kk * 128 + pp] = dd.astype(np.int16)
                dstc[sbi, pp, kk] = (dd - bb * 128).astype(np.float16)
        # wrap idx lists into [*, 128, X] dma_gather layout
        g1w = _wrap16(g1i.reshape(nsb * NSHARD, -1)).reshape(
            nsb, NSHARD, 128, sbn * tb * 8)
        # shard-major concat on free axis -> [nsb, 128, NSHARD * sbn*tb*8]
        g1w = np.concatenate([g1w[:, s] for s in range(NSHARD)], axis=2)
        g2w = _wrap16(g2i)                      # [nsb, 128, kpsb*8]
        packed.append({"g1i": np.ascontiguousarray(g1w),
                       "g2i": np.ascontiguousarray(g2w),
                       "dstc": dstc})
    return packed, tb


def _preprocess(cfg: Cfg, x, edge_index,
                W0, a_src0, a_dst0, b0, W1, a_src1, a_dst1, b1,
                W2, a_src2, a_dst2, b2):
    x = np.asarray(x, dtype=np.float32)
    edge_index = np.asarray(edge_index)
    loop = np.arange(cfg.n, dtype=np.int64)
    src = np.concatenate([np.asarray(edge_index[0]), loop]).astype(np.int64)
    dst = np.concatenate([np.asarray(edge_index[1]), loop]).astype(np.int64)

    packed, tb = _pack_edges(cfg, src, dst)

    hid, heads, ncls = cfg.hid, cfg.heads, cfg.ncls

    A0s = _block_diag_a(np.asarray(a_src0, np.float32))
    A0d = _block_diag_a(np.asarray(a_dst0, np.float32))
    A1s = _block_diag_a(np.asarray(a_src1, np.float32))
    A1d = _block_diag_a(np.asarray(a_dst1, np.float32))
    A2s = _block_diag_a(np.asarray(a_src2, np.float32))
    A2d = _block_diag_a(np.asarray(a_dst2, np.float32))

    W0 = np.asarray(W0, np.float32)
    W1 = np.asarray(W1, np.float32)
    W2 = np.asarray(W2, np.float32)

    W0ext = np.concatenate([W0, W0 @ A0s, W0 @ A0d], axis=1)          # [512,80]
    W1ext = np.concatenate([W1, W1 @ A1s, W1 @ A1d], axis=1)          # [64,80]
    W2ext = np.concatenate([np.eye(hid, dtype=np.float32),
                            W2 @ A2s, W2 @ A2d], axis=1)              # [64,80]
    W2stack = (np.ascontiguousarray(
        W2.reshape(hid, heads, ncls).transpose(1, 0, 2).reshape(heads * hid, ncls))
        / float(heads)).astype(np.float32)                            # [512,40]

    b0r = np.tile(np.asarray(b0, np.float32)[None, :], (128, 1))
    b1r = np.tile(np.asarray(b1, np.float32)[None, :], (128, 1))
    b2r = np.tile(np.asarray(b2, np.float32)[None, :], (128, 1))
    iota = np.tile(np.arange(128, dtype=np.float16)[None, :], (128, 1))

    in_maps = []
    for mm in range(cfg.m):
        xs = x[mm * cfg.npc:(mm + 1) * cfg.npc]
        xT = np.zeros((cfg.f_in, cfg.npad), dtype=np.float32)
        xT[:, :cfg.npc] = np.ascontiguousarray(xs.T)
        in_maps.append({
            "xT": xT,
            "g1i": packed[mm]["g1i"],
            "g2i": packed[mm]["g2i"],
            "dstc": packed[mm]["dstc"],
            "W0ext": W0ext, "W1ext": W1ext, "W2ext": W2ext,
            "W2stack": W2stack,
            "b0r": b0r, "b1r": b1r, "b2r": b2r,
            "iota": iota,
        })
    return in_maps, tb


# ----------------------------------------------------------------------------
# Device kernel
# ----------------------------------------------------------------------------

def _emit_kernel(ctx, tc, cfg: Cfg, io):
    import concourse.bass as bass
    import concourse.mybir as mybir
    from concourse.masks import make_identity

    nc = tc.nc
    f32 = mybir.dt.float32
    bf16 = mybir.dt.bfloat16
    f16 = mybir.dt.float16
    i16 = mybir.dt.int16
    AF = mybir.ActivationFunctionType
    OP = mybir.AluOpType
    B, TB, T, SB, FK = cfg.b, cfg.tb, cfg.t, cfg.sb, cfg.fk
    NSB, KPSB = cfg.nsb, cfg.kpsb
    heads, hid, ncls = cfg.heads, cfg.hid, cfg.ncls
    SHR = cfg.shard_rows
    rg = [list(range(cfg.m))]

    # ---- constants in SBUF ----
    cpool = ctx.enter_context(tc.tile_pool(name="consts", bufs=1))
    w0e = []
    for k in range(FK):
        tl = cpool.tile([128, 80], f32, tag=f"w0e{k}")
        nc.sync.dma_start(tl[:], io["W0ext"][k * 128:(k + 1) * 128, :])
        w0e.append(tl)
    w1e = cpool.tile([hid, 80], f32)
    nc.sync.dma_start(w1e[:], io["W1ext"][:])
    w2e = cpool.tile([hid, 80], f32)
    nc.sync.dma_start(w2e[:], io["W2ext"][:])
    w2s = []
    for k in range(4):
        tl = cpool.tile([128, ncls], f32, tag=f"w2s{k}")
        nc.sync.dma_start(tl[:], io["W2stack"][k * 128:(k + 1) * 128, :])
        w2s.append(tl)
    b0r = cpool.tile([128, hid], f32)
    nc.sync.dma_start(b0r[:], io["b0r"][:])
    b1r = cpool.tile([128, hid], f32)
    nc.sync.dma_start(b1r[:], io["b1r"][:])
    b2r = cpool.tile([128, ncls], f32)
    nc.sync.dma_start(b2r[:], io["b2r"][:])
    iota = cpool.tile([128, 128], f16)
    nc.sync.dma_start(iota[:], io["iota"][:])
    ident = cpool.tile([128, 128], f32)
    make_identity(nc, ident[:])

    # ---- internal DRAM: per-layer node tables ----
    hxp = [nc.dram_tensor(f"hxp{l}", [cfg.npad, ROWW], bf16) for l in range(3)]
    hxg = [nc.dram_tensor(f"hxg{l}", [cfg.m * cfg.npad, ROWW], bf16,
                          addr_space="Shared") for l in range(3)]
    dbg = io.get("dbg")

    # ------------------------------------------------------------------
    def store_hx(pool, ps_hxn, layer, bb):
        """psum [128, 80] f32 -> 256B table row [h bf16 | asrc f32 | adst f32]."""
        row = pool.tile([128, ROWW], bf16, tag="hxrow")
        nc.vector.tensor_copy(row[:, 0:hid], ps_hxn[:, 0:hid])
        nc.vector.tensor_copy(row[:, hid:hid + 32].bitcast(f32),
                              ps_hxn[:, hid:hid + 16])
        nc.vector.memset(row[:, hid + 32:ROWW], 0)
        nc.sync.dma_start(hxp[layer][bb * 128:(bb + 1) * 128, :], row[:])

    # ---- phase A: layer-0 node rows from x @ W0ext ----
    with tc.tile_pool(name="pa_sb", bufs=3) as pool, \
         tc.tile_pool(name="pa_ps", bufs=2, space="PSUM") as pps:
        for bb in range(B):
            ps = pps.tile([128, 80], f32, tag="hx0")
            for k in range(FK):
                xt = pool.tile([128, 128], f32, tag="xt")
                nc.sync.dma_start(
                    xt[:], io["xT"][k * 128:(k + 1) * 128, bb * 128:(bb + 1) * 128])
                nc.tensor.matmul(ps[:], xt[:], w0e[k][:],
                                 start=(k == 0), stop=(k == FK - 1))
            store_hx(pool, ps, 0, bb)

    if dbg is not None:
        nc.sync.dma_start(dbg["d_hx0loc"], hxp[0].ap())

    def fill_out():
        z = cpool.tile([128, ncls], mybir.dt.float32, tag="zfill")
        nc.vector.memset(z[:], 0)
        for bb in range(B):
            nc.sync.dma_start(io["out"][bb * 128:(bb + 1) * 128, :], z[:])

    if PHASES == "a":
        fill_out()
        return

    def allgather(l):
        nc.gpsimd.collective_compute(
            "AllGather", OP.bypass, replica_groups=rg,
            ins=[hxp[l].ap().opt()], outs=[hxg[l].ap().opt()])

    # ------------------------------------------------------------------
    def edge_gather(pool, l, sbi):
        """Superblock gathers + ST + w.  Returns (g1, st, w) tiles."""
        g1it = pool.tile([128, NSHARD * SB * TB * 8], i16, tag="g1i")
        nc.sync.dma_start(g1it[:], io["g1i"][sbi])
        g2it = pool.tile([128, KPSB * 8], i16, tag="g2i")
        nc.sync.dma_start(g2it[:], io["g2i"][sbi])
        dstc_t = pool.tile([128, KPSB], f16, tag="dstc")
        nc.sync.dma_start(dstc_t[:], io["dstc"][sbi])

        g1 = pool.tile([128, KPSB * ROWW], bf16, tag="g1")
        g1r = g1[:].rearrange("p (k c) -> p k c", c=ROWW)
        for s in range(NSHARD):
            nc.gpsimd.dma_gather(
                g1r[:, s * SB * TB:(s + 1) * SB * TB],
                hxg[l].ap()[s * SHR:(s + 1) * SHR, :],
                g1it[:, s * SB * TB * 8:(s + 1) * SB * TB * 8],
                SB * TB * 128,
                SB * TB * 128,
                ROWW,
                single_packet=False,
            )
        g2 = pool.tile([128, KPSB * ROWW], bf16, tag="g2")
        nc.gpsimd.dma_gather(
            g2[:].rearrange("p (k c) -> p k c", c=ROWW),
            hxp[l].ap(),
            g2it[:],
            KPSB * 128,
            KPSB * 128,
            ROWW,
            single_packet=False,
        )

        # one-hot (transposed) selection: st[p, k, d] = (dstc[p,k] == d)
        st = pool.tile([128, KPSB * 128], bf16, tag="st")
        nc.vector.tensor_tensor(
            out=st[:].rearrange("p (k d) -> p k d", d=128),
            in0=dstc_t[:].unsqueeze(2).to_broadcast([128, KPSB, 128]),
            in1=iota[:].unsqueeze(1).to_broadcast([128, KPSB, 128]),
            op=OP.is_equal)

        # e = asrc + adst; w = exp(leaky_relu(e))
        asrc = g1r[:, :, hid:hid + 32].bitcast(f32)[:, :, 0:heads]
        adst = g2[:].rearrange("p (k c) -> p k c", c=ROWW)[
            :, :, hid + 16:hid + 32].bitcast(f32)
        e_t = pool.tile([128, KPSB * heads], f32, tag="e")
        nc.vector.tensor_tensor(
            out=e_t[:].rearrange("p (k h) -> p k h", h=heads),
            in0=asrc, in1=adst, op=OP.add)
        lr = pool.tile([128, KPSB * heads], f32, tag="lr")
        nc.vector.tensor_scalar_mul(lr[:], e_t[:], NEG_SLOPE)
        nc.vector.tensor_tensor(out=lr[:], in0=lr[:], in1=e_t[:], op=OP.max)
        w_t = pool.tile([128, KPSB * heads], bf16, tag="w")
        nc.scalar.activation(w_t[:], lr[:], AF.Exp)

        if dbg is not None and l == 0 and sbi == 0:
            nc.sync.dma_start(dbg["d_glob0"], hxg[0].ap())
            nc.sync.dma_start(dbg["d_g1"], g1[:])
            nc.sync.dma_start(dbg["d_g2"], g2[:])
            nc.sync.dma_start(dbg["d_st"], st[:])
            nc.sync.dma_start(dbg["d_w"], w_t[:])
        return g1, st, w_t

    def blk_chunks(bi):
        """Canonical chunk ids belonging to block bi of a superblock."""
        return [s * (SB * TB) + bi * TB + j
                for s in range(NSHARD) for j in range(TB)]

    # ------------------------------------------------------------------
    def finalize_f(pool, ps_agg, brep):
        """num/den + bias + ELU -> fe [128, 64] f32."""
        recip = pool.tile([128, heads], f32, tag="recip")
        nc.vector.reciprocal(recip[:], ps_agg[:, hid:hid + heads])
        f_t = pool.tile([128, hid], f32, tag="f")
        nc.vector.tensor_tensor(
            out=f_t[:].rearrange("p (h c) -> p h c", c=C_HID),
            in0=ps_agg[:, 0:hid].rearrange("p (h c) -> p h c", c=C_HID),
            in1=recip[:].unsqueeze(2).to_broadcast([128, heads, C_HID]),
            op=OP.mult)
        nc.vector.tensor_tensor(out=f_t[:], in0=f_t[:], in1=brep[:], op=OP.add)
        xneg = pool.tile([128, hid], f32, tag="xneg")
        nc.vector.tensor_scalar_min(xneg[:], f_t[:], 0.0)
        expn = pool.tile([128, hid], f32, tag="expn")
        nc.scalar.activation(expn[:], xneg[:], AF.Exp)
        xpos = pool.tile([128, hid], f32, tag="xpos")
        nc.vector.tensor_scalar_max(xpos[:], f_t[:], 0.0)
        fe = pool.tile([128, hid], f32, tag="fe")
        nc.vector.tensor_tensor(out=fe[:], in0=expn[:], in1=xpos[:], op=OP.add)
        nc.vector.tensor_scalar_add(fe[:], fe[:], -1.0)
        return fe

    if PHASES == "ag":
        allgather(0)
        allgather(1)
        allgather(2)
        fill_out()
        return

    if PHASES in ("b0g", "b0gg", "b0w"):
        allgather(0)
        with tc.tile_pool(name="bg_sb", bufs=2) as pool:
            for sbi in range(NSB):
                if PHASES == "b0w":
                    edge_gather(pool, 0, sbi)
                else:
                    g1it = pool.tile([128, NSHARD * SB * TB * 8], i16, tag="g1i")
                    nc.sync.dma_start(g1it[:], io["g1i"][sbi])
                    g2it = pool.tile([128, KPSB * 8], i16, tag="g2i")
                    nc.sync.dma_start(g2it[:], io["g2i"][sbi])
                    g1 = pool.tile([128, KPSB * ROWW], bf16, tag="g1")
                    g1r = g1[:].rearrange("p (k c) -> p k c", c=ROWW)
                    for s in range(NSHARD):
                        nc.gpsimd.dma_gather(
                            g1r[:, s * SB * TB:(s + 1) * SB * TB],
                            hxg[0].ap()[s * SHR:(s + 1) * SHR, :],
                            g1it[:, s * SB * TB * 8:(s + 1) * SB * TB * 8],
                            SB * TB * 128, SB * TB * 128, ROWW,
                            single_packet=False)
                    if PHASES == "b0gg":
                        g2 = pool.tile([128, KPSB * ROWW], bf16, tag="g2")
                        nc.gpsimd.dma_gather(
                            g2[:].rearrange("p (k c) -> p k c", c=ROWW),
                            hxp[0].ap(), g2it[:],
                            KPSB * 128, KPSB * 128, ROWW,
                            single_packet=False)
        fill_out()
        return

    # ---- phases B0/B1: edge aggregation for layers 0,1 -> hx1, hx2 ----
    for l in range(1 if PHASES == "b0" else 2):
        allgather(l)
        wnext = w1e if l == 0 else w2e
        brep = b0r if l == 0 else b1r
        with tc.tile_pool(name=f"pb{l}_sb", bufs=2) as pool, \
             tc.tile_pool(name=f"pb{l}_ps", bufs=2, space="PSUM") as pps:
            for sbi in range(NSB):
                g1, st, w_t = edge_gather(pool, l, sbi)
                g1r = g1[:].rearrange("p (k c) -> p k c", c=ROWW)
                wr = w_t[:].rearrange("p (k h) -> p k h", h=heads)
                for bi in range(SB):
                    bb = sbi * SB + bi
                    # rhs [128, T*72] = [w*G | w] per chunk (s-major order)
                    rhs = pool.tile([128, T * 72], bf16, tag="rhs")
                    rhs_r = rhs[:].rearrange("p (k c) -> p k c", c=72)
                    wrb = wr.rearrange("p (s b j) h -> p s b j h",
                                       s=NSHARD, b=SB)
                    g1b = g1r[:, :, 0:hid].rearrange(
                        "p (s b j) (h c) -> p s b j h c", s=NSHARD, b=SB,
                        c=C_HID)
                    for s in range(NSHARD):
                        nc.vector.tensor_tensor(
                            out=rhs_r[:, s * TB:(s + 1) * TB, 0:hid].rearrange(
                                "p j (h c) -> p j h c", c=C_HID),
                            in0=wrb[:, s, bi].unsqueeze(3)
                                .to_broadcast([128, TB, heads, C_HID]),
                            in1=g1b[:, s, bi],
                            op=OP.mult)
                    nc.vector.tensor_copy(
                        rhs_r[:, :, hid:72].rearrange(
                            "p (s j) h -> p s j h", s=NSHARD),
                        wrb[:, :, bi])

                    ps_agg = pps.tile([128, 72], f32, tag="agg")
                    chunks = blk_chunks(bi)
                    for ci, kk in enumerate(chunks):
                        nc.tensor.matmul(
                            ps_agg[:],
                            st[:, kk * 128:(kk + 1) * 128],
                            rhs[:, ci * 72:(ci + 1) * 72],
                            start=(ci == 0), stop=(ci == len(chunks) - 1))

                    if dbg is not None and l == 0 and bb == 0:
                        agg_sb = pool.tile([128, 72], f32, tag="dbg_agg")
                        nc.vector.tensor_copy(agg_sb[:], ps_agg[:])
                        nc.sync.dma_start(dbg["d_agg"], agg_sb[:])

                    fe = finalize_f(pool, ps_agg, brep)
                    if dbg is not None and l == 0 and bb == 0:
                        nc.sync.dma_start(dbg["d_fe"], fe[:])

                    ps_ft = pps.tile([hid, 128], f32, tag="ft")
                    nc.tensor.transpose(ps_ft[:], fe[:], ident[:])
                    ft = pool.tile([hid, 128], f32, tag="ftsb")
                    nc.vector.tensor_copy(ft[:], ps_ft[:])
                    ps_hxn = pps.tile([128, 80], f32, tag="hxn")
                    nc.tensor.matmul(ps_hxn[:], ft[:], wnext[:],
                                     start=True, stop=True)
                    store_hx(pool, ps_hxn, l + 1, bb)

    if PHASES == "b0":
        fill_out()
        return

    # ---- phase C: layer-2 edge aggregation + classifier head ----
    allgather(2)
    with tc.tile_pool(name="pc_sb", bufs=2) as pool, \
         tc.tile_pool(name="pc_ps", bufs=1, space="PSUM") as pps, \
         tc.tile_pool(name="pc_ps2", bufs=2, space="PSUM") as pps2:
        for sbi in range(NSB):
            g1, st, w_t = edge_gather(pool, 2, sbi)
            g1r = g1[:].rearrange("p (k c) -> p k c", c=ROWW)
            wr = w_t[:].rearrange("p (k h) -> p k h", h=heads)
            for bi in range(SB):
                bb = sbi * SB + bi
                # wg8 [128, T*512]: per head h, w[:,k,h] * G[:,k,:]
                wg8 = pool.tile([128, T * heads * hid], bf16, tag="wg8")
                wg8r = wg8[:].rearrange("p (k h c) -> p k h c", h=heads, c=hid)
                wrb = wr.rearrange("p (s b j) h -> p s b j h", s=NSHARD, b=SB)
                g1c = g1r[:, :, 0:hid].rearrange(
                    "p (s b j) c -> p s b j c", s=NSHARD, b=SB)
                for s in range(NSHARD):
                    nc.vector.tensor_tensor(
                        out=wg8r[:, s * TB:(s + 1) * TB],
                        in0=wrb[:, s, bi].unsqueeze(3)
                            .to_broadcast([128, TB, heads, hid]),
                        in1=g1c[:, s, bi].unsqueeze(2)
                            .to_broadcast([128, TB, heads, hid]),
                        op=OP.mult)
                wloc = pool.tile([128, T * heads], bf16, tag="wloc")
                nc.vector.tensor_copy(
                    wloc[:].rearrange("p (s j h) -> p s j h", s=NSHARD, j=TB),
                    wrb[:, :, bi])

                ps_num = pps.tile([128, heads * hid], f32, tag="num")
                ps_den = pps.tile([128, heads], f32, tag="den")
                chunks = blk_chunks(bi)
                for ci, kk in enumerate(chunks):
                    lhsT = st[:, kk * 128:(kk + 1) * 128]
                    nc.tensor.matmul(
                        ps_num[:], lhsT,
                        wg8[:, ci * heads * hid:(ci + 1) * heads * hid],
                        start=(ci == 0), stop=(ci == len(chunks) - 1))
                    nc.tensor.matmul(
                        ps_den[:], lhsT,
                        wloc[:, ci * heads:(ci + 1) * heads],
                        start=(ci == 0), stop=(ci == len(chunks) - 1))

                recip = pool.tile([128, heads], f32, tag="recip2")
                nc.vector.reciprocal(recip[:], ps_den[:])
                fnum = pool.tile([128, heads * hid], f32, tag="fnum")
                nc.vector.tensor_tensor(
                    out=fnum[:].rearrange("p (h c) -> p h c", c=hid),
                    in0=ps_num[:].rearrange("p (h c) -> p h c", c=hid),
                    in1=recip[:].unsqueeze(2).to_broadcast([128, heads, hid]),
                    op=OP.mult)

                ps_o = pps2.tile([128, ncls], f32, tag="o")
                for k in range(4):
                    ps_fT = pps2.tile([128, 128], f32, tag="fT")
                    nc.tensor.transpose(
                        ps_fT[:], fnum[:, k * 128:(k + 1) * 128], ident[:])
                    fT = pool.tile([128, 128], f32, tag="fTsb")
                    nc.vector.tensor_copy(fT[:], ps_fT[:])
                    nc.tensor.matmul(ps_o[:], fT[:], w2s[k][:],
                                     start=(k == 0), stop=(k == 3))

                o_t = pool.tile([128, ncls], f32, tag="o_sb")
                nc.vector.tensor_tensor(out=o_t[:], in0=ps_o[:], in1=b2r[:],
                                        op=OP.add)
                mx = pool.tile([128, 1], f32, tag="mx")
                nc.vector.tensor_reduce(mx[:], o_t[:],
                                        axis=mybir.AxisListType.X, op=OP.max)
                os_t = pool.tile([128, ncls], f32, tag="os")
                nc.vector.tensor_tensor(out=os_t[:], in0=o_t[:],
                                        in1=mx[:].to_broadcast([128, ncls]),
                                        op=OP.subtract)
                ex = pool.tile([128, ncls], f32, tag="ex")
                ssum = pool.tile([128, 1], f32, tag="ssum")
                nc.scalar.activation(ex[:], os_t[:], AF.Exp, accum_out=ssum[:])
                lse = pool.tile([128, 1], f32, tag="lse")
                nc.scalar.activation(lse[:], ssum[:], AF.Ln)
                res = pool.tile([128, ncls], f32, tag="res")
                nc.vector.tensor_tensor(out=res[:], in0=os_t[:],
                                        in1=lse[:].to_broadcast([128, ncls]),
                                        op=OP.subtract)
                nc.sync.dma_start(io["out"][bb * 128:(bb + 1) * 128, :], res[:])


# ----------------------------------------------------------------------------
# Program build + run
# ----------------------------------------------------------------------------

_NC_CACHE = {}


def _build_program(cfg: Cfg, debug=False):
    from contextlib import ExitStack
    import concourse.tile as tile
    import concourse.mybir as mybir
    from concourse import bacc

    key = (cfg, debug, PHASES)
    if key in _NC_CACHE:
        return _NC_CACHE[key]

    f32 = mybir.dt.float32
    f16 = mybir.dt.float16
    i16 = mybir.dt.int16
    nc = bacc.Bacc("TRN2", target_bir_lowering=False, debug=False,
                   num_devices=cfg.m)

    io = {}
    def inp(name, shape, dtype):
        io[name] = nc.dram_tensor(name, list(shape), dtype,
                                  kind="ExternalInput").ap()
    inp("xT", (cfg.f_in, cfg.npad), f32)
    inp("g1i", (cfg.nsb, 128, NSHARD * cfg.sb * cfg.tb * 8), i16)
    inp("g2i", (cfg.nsb, 128, cfg.kpsb * 8), i16)
    inp("dstc", (cfg.nsb, 128, cfg.kpsb), f16)
    inp("W0ext", (cfg.f_in, 80), f32)
    inp("W1ext", (cfg.hid, 80), f32)
    inp("W2ext", (cfg.hid, 80), f32)
    inp("W2stack", (cfg.heads * cfg.hid, cfg.ncls), f32)
    inp("b0r", (128, cfg.hid), f32)
    inp("b1r", (128, cfg.hid), f32)
    inp("b2r", (128, cfg.ncls), f32)
    inp("iota", (128, 128), f16)
    io["out"] = nc.dram_tensor("out", [cfg.npad, cfg.ncls], f32,
                               kind="ExternalOutput").ap()

    if debug:
        bf16 = mybir.dt.bfloat16
        dbg_specs = {
            "d_hx0loc": ([cfg.npad, ROWW], bf16),
            "d_glob0": ([cfg.m * cfg.npad, ROWW], bf16),
            "d_g1": ([128, cfg.kpsb * ROWW], bf16),
            "d_g2": ([128, cfg.kpsb * ROWW], bf16),
            "d_st": ([128, cfg.kpsb * 128], bf16),
            "d_w": ([128, cfg.kpsb * cfg.heads], bf16),
            "d_agg": ([128, 72], f32),
            "d_fe": ([128, cfg.hid], f32),
        }
        io["dbg"] = {
            name: nc.dram_tensor(name, list(shape), dt_,
                                 kind="ExternalOutput").ap()
            for name, (shape, dt_) in dbg_specs.items()
        }

    with tile.TileContext(nc) as tc:
        with ExitStack() as ctx:
            _emit_kernel(ctx, tc, cfg, io)
    nc.compile()
    _NC_CACHE[key] = nc
    return nc


class _SimResults:
    def __init__(self, results):
        self.results = results
        self.exec_time_ns = None


def run(cfg: Cfg, inputs, trace=False, debug=False, sim=False):
    in_maps, tb = _preprocess(cfg, **inputs)
    if tb != cfg.tb:
        cfg = Cfg(**{**cfg.__dict__, "tb": tb})
        in_maps, tb = _preprocess(cfg, **inputs)
    nc = _build_program(cfg, debug=debug)
    if sim:
        from concourse.bass_interp import MultiCoreSim
        ms = MultiCoreSim(nc, num_cores=cfg.m, trace=False,
                          require_finite=False, require_nnan=False)
        for mm in range(cfg.m):
            for k, v in in_maps[mm].items():
                ms.cores[mm].tensor(k)[:] = v
        ms.simulate(check_with_hw=False)
        names = [t for t in ("out",)]
        results = []
        for mm in range(cfg.m):
            r = {"out": np.array(ms.cores[mm].mem_tensor("out"))}
            if debug:
                for nme in ("d_hx0loc", "d_glob0", "d_g1", "d_g2", "d_st",
                            "d_w", "d_agg", "d_fe"):
                    r[nme] = np.array(ms.cores[mm].mem_tensor(nme))
            results.append(r)
        res = _SimResults(results)
    else:
        from concourse.bass_utils import run_bass_kernel_spmd
        res = run_bass_kernel_spmd(nc, in_maps, list(range(cfg.m)),
                                   trace=trace)
    out = np.concatenate(
        [res.results[mm]["out"][:cfg.npc] for mm in range(cfg.m)], axis=0)
    return out.astype(np.float32), res, cfg


def make_cfg(n=N, f_in=F_IN, ncls=N_CLS, m=M_CORES, tb=1, sb=2):
    npc = n // m
    b = int(math.ceil(npc / 128))
    assert b % sb == 0, (b, sb)
    return Cfg(m=m, n=n, f_in=f_in, ncls=ncls, npc=npc, b=b, tb=tb, sb=sb)


def kernel(**inputs) -> np.ndarray:
    cfg = make_cfg()
    out, _, _ = run(cfg, inputs)
    return out



# revision 12
# speedup vs baseline: 1.2593x; 1.2593x over previous
"""GAT (3-layer, PyG-style) on 8 Trainium2 NeuronCores via Bass/Tile — v2.

Sharding: nodes split across 8 cores by destination range (12500/core,
padded to 12544 = 98*128).  Edges (incl. self-loops) partitioned by dst
owner, bucketed by (dst block, src shard) with VARIABLE per-bucket tile
counts (chunks of 128 edge slots), packed host-side.

Per layer each core computes fused node rows
  row[n] = [h c-major bf16 x64 | ones x8 | asrc bf16 x8 | pad] (256 B)
with PE matmuls, AllGathers the table, then per gather-superblock (7 dst
blocks): dma_gathers edge source rows (4 src shards, calls capped at
~2560 idx), builds one-hot st (edge->dst) AND transposed one-hot stT
(via partition-broadcast DMA of transposed dst offsets), fetches per-edge
adst with small stT x adst_stash matmuls (NO second gather), computes
w = exp(leaky_relu(asrc+adst)), builds rhs = w * row[0:72] in a single
broadcast multiply (ones columns make [w*G | w]), and aggregates num/den
per dst block with PE one-hot matmuls in PSUM.  out = num/den equals the
reference's stabilized softmax exactly (in exact arithmetic).
Layer 2 keeps per-head numerators (8x64 in PSUM) and applies W2 /
head-mean / log_softmax after normalization.
"""

import math
from dataclasses import dataclass

import numpy as np
import ml_dtypes

# ----------------------------------------------------------------------------
# Problem constants (hardcoded per the harness contract)
# ----------------------------------------------------------------------------
N = 100000
F_IN = 512
H = 8
C_HID = 8
N_CLS = 40
E = 1600000
NEG_SLOPE = 0.2
M_CORES = 8
HID = H * C_HID  # 64
NSHARD = 4       # src shards so gather indices fit int16
ROWW = 128       # table row width in bf16 elements (256 B)
GCAP = 20        # max chunks (x128 idx) per dma_gather call (2560 idx)
SENT = 30000.0   # dst-offset sentinel for pad slots


@dataclass(frozen=True)
class Cfg:
    m: int          # cores
    n: int          # total nodes
    f_in: int       # input features (multiple of 128)
    ncls: int       # classes
    npc: int        # nodes per core
    b: int          # dst blocks per core (npc padded to b*128)
    gsb: int        # dst blocks per gather superblock
    hid: int = 64
    heads: int = 8

    @property
    def npad(self):
        return self.b * 128

    @property
    def fk(self):
        return self.f_in // 128

    @property
    def ngsb(self):
        return self.b // self.gsb

    @property
    def shard_rows(self):
        return self.m * self.npad // NSHARD


class Meta:
    """Static (data-dependent, core-independent) layout of edge chunks."""

    def __init__(self, cfg: Cfg, nch: np.ndarray):
        # nch: [b, NSHARD] chunks per bucket (max over cores)
        self.nch = nch
        gsb, ngsb = cfg.gsb, cfg.ngsb
        self.K = np.zeros(ngsb, dtype=np.int64)          # chunks per gsb
        self.regions = []   # [g] -> list of (s, c0, rc)
        self.bucket_c0 = np.zeros((cfg.b, NSHARD), dtype=np.int64)  # gsb-local
        self.block_chunks = [[] for _ in range(cfg.b)]   # gsb-local chunk ids
        self.chunk_block = []                            # [g][ck] -> local blk
        for g in range(ngsb):
            co = 0
            regs = []
            cb = []
            for s in range(NSHARD):
                c0 = co
                for bl in range(g * gsb, (g + 1) * gsb):
                    self.bucket_c0[bl, s] = co
                    nb = int(nch[bl, s])
                    self.block_chunks[bl].extend(range(co, co + nb))
                    cb.extend([bl - g * gsb] * nb)
                    co += nb
                regs.append((s, c0, co - c0))
            self.regions.append(regs)
            self.chunk_block.append(cb)
            self.K[g] = co
        self.KMAX = int(self.K.max())
        self.RCMAX = max(max(rc for _, _, rc in regs) for regs in self.regions)
        # contiguous runs of each block's chunks (one per shard) + max chunks
        self.block_runs = []
        self.TBMAX = 1
        for bl in range(cfg.b):
            runs = []
            for s in range(NSHARD):
                nb = int(nch[bl, s])
                if nb:
                    runs.append((int(self.bucket_c0[bl, s]), nb))
            self.block_runs.append(runs)
            self.TBMAX = max(self.TBMAX, int(nch[bl].sum()))
        # gather call list: [g][s] -> list of (chunk_start(gsb-local), nchunks)
        self.calls = []
        for g in range(ngsb):
            percall = []
            for s, c0, rc in self.regions[g]:
                lst = []
                nc_ = int(math.ceil(rc / GCAP)) if rc else 0
                off = c0
                for i in range(nc_):
                    take = (rc + nc_ - 1 - i) // nc_
                    lst.append((off, take))
                    off += take
                percall.append(lst)
            self.calls.append(percall)

    def key(self):
        return (self.nch.tobytes(), self.KMAX, self.RCMAX)


# ----------------------------------------------------------------------------
# Host-side preprocessing
# ----------------------------------------------------------------------------

def _block_diag_a(a):
    Hh, Cc = a.shape
    out = np.zeros((Hh * Cc, Hh), dtype=np.float32)
    for h in range(Hh):
        out[h * Cc:(h + 1) * Cc, h] = a[h]
    return out


def _cmajor_cols(W):
    F = W.shape[0]
    return np.ascontiguousarray(
        W.reshape(F, H, C_HID).transpose(0, 2, 1).reshape(F, HID))


def _cmajor_rows(W):
    K = W.shape[1]
    return np.ascontiguousarray(
        W.reshape(H, C_HID, K).transpose(1, 0, 2).reshape(HID, K))


def _wrap16(flat):
    """[n] int16 idx -> [128, n/16] dma_gather layout."""
    n = flat.shape[0]
    assert n % 16 == 0
    arr = flat.reshape(n // 16, 16).T          # [16, n//16]
    return np.tile(arr, (8, 1)).astype(np.int16)


def _pack_edges(cfg: Cfg, src, dst):
    m, npc, b, gsb = cfg.m, cfg.npc, cfg.b, cfg.gsb
    npad, ngsb = cfg.npad, cfg.ngsb
    shard_rows = cfg.shard_rows
    percore = []
    cnts = np.zeros((m, b, NSHARD), dtype=np.int64)
    for mm in range(m):
        sel = np.nonzero(dst // npc == mm)[0]
        dl = (dst[sel] - mm * npc).astype(np.int64)
        sg = src[sel]
        sg = (sg // npc) * npad + (sg % npc)
        blk = dl // 128
        shard = sg // shard_rows
        order = np.lexsort((blk, shard, blk // gsb))
        dl, sg, blk, shard = dl[order], sg[order], blk[order], shard[order]
        np.add.at(cnts[mm], (blk, shard), 1)
        percore.append((sg, dl, blk, shard))
    nch = np.ceil(cnts.max(axis=0) / 128).astype(np.int64)   # [b, NSHARD]
    meta = Meta(cfg, nch)

    packed = []
    for mm in range(m):
        sg, dl, blk, shard = percore[mm]
        cnt = cnts[mm]
        g1i = np.zeros((ngsb, 128, meta.KMAX * 8), dtype=np.int16)
        dstc = np.full((ngsb, 128, meta.KMAX), SENT, dtype=np.float16)
        dstcT = np.full((ngsb, 1, meta.KMAX * 128), SENT, dtype=np.float16)
        flat = np.zeros((ngsb, meta.KMAX * 128), dtype=np.int16)
        pos = 0
        for g in range(ngsb):
            for s in range(NSHARD):
                for bl in range(g * gsb, (g + 1) * gsb):
                    c = int(cnt[bl, s])
                    if c == 0:
                        continue
                    co = int(meta.bucket_c0[bl, s])
                    q = np.arange(c)
                    ss = (sg[pos:pos + c] - s * shard_rows).astype(np.int16)
                    dd = (dl[pos:pos + c] - bl * 128).astype(np.float16)
                    pos += c
                    flat[g, co * 128 + q] = ss
                    dstc[g, q % 128, co + q // 128] = dd
                    dstcT[g, 0, co * 128 + q] = dd
            # wrap idx per call segment
            for s, lst in zip(range(NSHARD), meta.calls[g]):
                for (a, nchk) in lst:
                    seg = flat[g, a * 128:(a + nchk) * 128]
                    g1i[g][:, a * 8:(a + nchk) * 8] = _wrap16(seg)
        assert pos == len(sg)
        packed.append({"g1i": g1i, "dstc": dstc, "dstcT": dstcT})
    return packed, meta


def _preprocess(cfg: Cfg, x, edge_index,
                W0, a_src0, a_dst0, b0, W1, a_src1, a_dst1, b1,
                W2, a_src2, a_dst2, b2):
    x = np.asarray(x, dtype=np.float32)
    edge_index = np.asarray(edge_index)
    # the implicit self-loop the reference appends is handled locally
    # on-device; original edges (incl. any explicit self-edges) are gathered
    src = np.asarray(edge_index[0]).astype(np.int64)
    dst = np.asarray(edge_index[1]).astype(np.int64)

    packed, meta = _pack_edges(cfg, src, dst)

    hid, heads, ncls = cfg.hid, cfg.heads, cfg.ncls

    W0 = np.asarray(W0, np.float32)
    W1 = np.asarray(W1, np.float32)
    W2 = np.asarray(W2, np.float32)
    A0s = _block_diag_a(np.asarray(a_src0, np.float32))
    A0d = _block_diag_a(np.asarray(a_dst0, np.float32))
    A1s = _block_diag_a(np.asarray(a_src1, np.float32))
    A1d = _block_diag_a(np.asarray(a_dst1, np.float32))
    A2s = _block_diag_a(np.asarray(a_src2, np.float32))
    A2d = _block_diag_a(np.asarray(a_dst2, np.float32))

    W1p = _cmajor_rows(W1)
    W2p = _cmajor_rows(W2)

    W0ext = np.concatenate([_cmajor_cols(W0), W0 @ A0s, W0 @ A0d], axis=1)
    W1ext = np.concatenate([_cmajor_cols(W1p), W1p @ A1s, W1p @ A1d], axis=1)
    W2ext = np.concatenate([np.eye(hid, dtype=np.float32),
                            W2p @ A2s, W2p @ A2d], axis=1)
    W2stack = (np.ascontiguousarray(
        W2p.reshape(hid, heads, ncls).transpose(1, 0, 2)
        .reshape(heads * hid, ncls)) / float(heads)).astype(np.float32)

    b0cm = np.asarray(b0, np.float32).reshape(H, C_HID).T.reshape(-1)
    b1cm = np.asarray(b1, np.float32).reshape(H, C_HID).T.reshape(-1)
    b0r = np.tile(b0cm[None, :], (128, 1)).astype(np.float32)
    b1r = np.tile(b1cm[None, :], (128, 1)).astype(np.float32)
    b2r = np.tile(np.asarray(b2, np.float32)[None, :], (128, 1))
    iota = np.tile(np.arange(128, dtype=np.float16)[None, :], (128, 1))
    iota_col = np.arange(128, dtype=np.float16).reshape(128, 1)

    bf16 = ml_dtypes.bfloat16
    W0e_bf = W0ext.astype(bf16)

    in_maps = []
    for mm in range(cfg.m):
        xs = x[mm * cfg.npc:(mm + 1) * cfg.npc]
        xT = np.zeros((cfg.f_in, cfg.npad), dtype=bf16)
        xT[:, :cfg.npc] = np.ascontiguousarray(xs.T).astype(bf16)
        in_maps.append({
            "xT": xT,
            "g1i": packed[mm]["g1i"],
            "dstc": packed[mm]["dstc"],
            "dstcT": packed[mm]["dstcT"],
            "W0ext": W0e_bf, "W1ext": W1ext, "W2ext": W2ext,
            "W2stack": W2stack,
            "b0r": b0r, "b1r": b1r, "b2r": b2r,
            "iota": iota, "iota_col": iota_col,
        })
    return in_maps, meta


# ----------------------------------------------------------------------------
# Device kernel
# ----------------------------------------------------------------------------

def _emit_kernel(ctx, tc, cfg: Cfg, meta: Meta, io):
    import concourse.mybir as mybir
    from concourse.masks import make_identity

    nc = tc.nc
    f32 = mybir.dt.float32
    bf16 = mybir.dt.bfloat16
    f16 = mybir.dt.float16
    i16 = mybir.dt.int16
    AF = mybir.ActivationFunctionType
    OP = mybir.AluOpType
    B, GSB, NGSB, FK = cfg.b, cfg.gsb, cfg.ngsb, cfg.fk
    heads, hid, ncls = cfg.heads, cfg.hid, cfg.ncls
    SHR = cfg.shard_rows
    KMAX, RCMAX = meta.KMAX, meta.RCMAX
    rg = [list(range(cfg.m))]

    # ---- constants / persistent state in SBUF ----
    cpool = ctx.enter_context(tc.tile_pool(name="consts", bufs=1))
    w0e = []
    for k in range(FK):
        tl = cpool.tile([128, 80], bf16, tag=f"w0e{k}")
        nc.sync.dma_start(tl[:], io["W0ext"][k * 128:(k + 1) * 128, :])
        w0e.append(tl)
    w1e = cpool.tile([hid, 80], f32)
    nc.sync.dma_start(w1e[:], io["W1ext"][:])
    w2e = cpool.tile([hid, 80], f32)
    nc.sync.dma_start(w2e[:], io["W2ext"][:])
    w2s = []
    for k in range(4):
        tl = cpool.tile([128, ncls], f32, tag=f"w2s{k}")
        nc.sync.dma_start(tl[:], io["W2stack"][k * 128:(k + 1) * 128, :])
        w2s.append(tl)
    b0r = cpool.tile([128, hid], f32)
    nc.sync.dma_start(b0r[:], io["b0r"][:])
    b1r = cpool.tile([128, hid], f32)
    nc.sync.dma_start(b1r[:], io["b1r"][:])
    b2r = cpool.tile([128, ncls], f32)
    nc.sync.dma_start(b2r[:], io["b2r"][:])
    iota = cpool.tile([128, 128], f16)
    nc.sync.dma_start(iota[:], io["iota"][:])
    iota_col = cpool.tile([128, 1], f16)
    nc.sync.dma_start(iota_col[:], io["iota_col"][:])
    ident = cpool.tile([128, 128], f32)
    make_identity(nc, ident[:])
    stash = []
    for i in range(2):
        stash_t = cpool.tile([128, B * heads], bf16, tag=f"stash{i}")
        stash.append(stash_t)

    # ---- internal DRAM: per-layer node tables ----
    hxp = [nc.dram_tensor(f"hxp{l}", [cfg.npad, ROWW], bf16) for l in range(3)]
    hxg = [nc.dram_tensor(f"hxg{l}", [cfg.m * cfg.npad, ROWW], bf16,
                          addr_space="Shared") for l in range(3)]
    dbg = io.get("dbg")

    # ------------------------------------------------------------------
    def store_hx(pool, ps, layer, bb):
        """psum [128,80] = [h cm 64 | asrc 8 | adst 8] -> 256B row + stash."""
        row = pool.tile([128, ROWW], bf16, tag="hxrow")
        nc.vector.tensor_copy(row[:, 0:hid], ps[:, 0:hid])
        nc.vector.memset(row[:, hid:hid + 8], 1.0)
        nc.vector.tensor_copy(row[:, hid + 8:hid + 16], ps[:, hid:hid + 8])
        nc.vector.memset(row[:, hid + 16:ROWW], 0)
        nc.sync.dma_start(hxp[layer][bb * 128:(bb + 1) * 128, :], row[:])
        nc.vector.tensor_copy(
            stash[layer % 2][:, bb * heads:(bb + 1) * heads],
            ps[:, hid + 8:hid + 16])

    # ---- phase A: layer-0 node rows from x @ W0ext ----
    with tc.tile_pool(name="pa_sb", bufs=3) as pool, \
         tc.tile_pool(name="pa_ps", bufs=2, space="PSUM") as pps:
        for bb in range(B):
            ps = pps.tile([128, 80], f32, tag="hx0")
            for k in range(FK):
                xt = pool.tile([128, 128], bf16, tag="xt")
                nc.sync.dma_start(
                    xt[:], io["xT"][k * 128:(k + 1) * 128,
                                    bb * 128:(bb + 1) * 128])
                nc.tensor.matmul(ps[:], xt[:], w0e[k][:],
                                 start=(k == 0), stop=(k == FK - 1))
            store_hx(pool, ps, 0, bb)

    def allgather(l):
        nc.gpsimd.collective_compute(
            "AllGather", OP.bypass, replica_groups=rg,
            ins=[hxp[l].ap().opt()], outs=[hxg[l].ap().opt()])

    # ------------------------------------------------------------------
    # main per-layer edge-aggregation loops
    for l in range(3):
        allgather(l)
        last = (l == 2)
        wnext = w1e if l == 0 else w2e
        brep = b0r if l == 0 else b1r
        with tc.tile_pool(name=f"pb{l}_g", bufs=3) as pg, \
             tc.tile_pool(name=f"pb{l}_1", bufs=2) as p1, \
             tc.tile_pool(name=f"pb{l}_c", bufs=2) as pc, \
             tc.tile_pool(name=f"pb{l}_ps", bufs=2, space="PSUM") as pps, \
             tc.tile_pool(name=f"pb{l}_ps1", bufs=(1 if last else 2),
                          space="PSUM") as pps1:
            for g in range(NGSB):
                K = int(meta.K[g])
                it = pg.tile([128, KMAX * 8], i16, tag="g1i")
                nc.sync.dma_start(it[:, :K * 8], io["g1i"][g][:, :K * 8])
                dc = pg.tile([128, KMAX], f16, tag="dstc")
                nc.sync.dma_start(dc[:, :K], io["dstc"][g][:, :K])
                g1 = pg.tile([128, KMAX * ROWW], bf16, tag="g1")
                g1r = g1[:].rearrange("p (k c) -> p k c", c=ROWW)
                for s, lst in zip(range(NSHARD), meta.calls[g]):
                    src_ap = hxg[l].ap()[s * SHR:(s + 1) * SHR, :]
                    for (a, nchk) in lst:
                        nc.gpsimd.dma_gather(
                            g1r[:, a:a + nchk], src_ap,
                            it[:, a * 8:(a + nchk) * 8],
                            nchk * 128, nchk * 128, ROWW,
                            single_packet=False)
                st = p1.tile([128, KMAX * 128], bf16, tag="st")
                nc.vector.tensor_tensor(
                    out=st[:, :K * 128].rearrange("p (k d) -> p k d", d=128),
                    in0=dc[:, :K].unsqueeze(2).to_broadcast([128, K, 128]),
                    in1=iota[:].unsqueeze(1).to_broadcast([128, K, 128]),
                    op=OP.is_equal)
                w_t = pg.tile([128, KMAX * heads], bf16, tag="w")
                if not last:
                    rhs = pc.tile([128, KMAX * 72], bf16, tag="rhs")
                for s, c0, rc in meta.regions[g]:
                    if rc == 0:
                        continue
                    dT = p1.tile([128, RCMAX * 128], f16, tag="dstcT")
                    nc.sync.dma_start(
                        dT[:, :rc * 128],
                        io["dstcT"][g][:, c0 * 128:(c0 + rc) * 128]
                        .to_broadcast([128, rc * 128]))
                    sT = p1.tile([128, RCMAX * 128], bf16, tag="stT")
                    nc.vector.tensor_tensor(
                        out=sT[:, :rc * 128],
                        in0=dT[:, :rc * 128],
                        in1=iota_col[:].to_broadcast([128, rc * 128]),
                        op=OP.is_equal)
                    ps_adst = pps1.tile([128, RCMAX * heads], f32, tag="adst")
                    for ck in range(rc):
                        bloc = meta.chunk_block[g][c0 + ck]
                        bb = g * GSB + bloc
                        nc.tensor.matmul(
                            ps_adst[:, ck * heads:(ck + 1) * heads],
                            sT[:, ck * 128:(ck + 1) * 128],
                            stash[l % 2][:, bb * heads:(bb + 1) * heads],
                            start=True, stop=True)
                    e_t = pc.tile([128, RCMAX * heads], f32, tag="e")
                    nc.vector.tensor_tensor(
                        out=e_t[:, :rc * heads].rearrange(
                            "p (k h) -> p k h", h=heads),
                        in0=g1r[:, c0:c0 + rc, hid + 8:hid + 16],
                        in1=ps_adst[:, :rc * heads].rearrange(
                            "p (k h) -> p k h", h=heads),
                        op=OP.add)
                    lr = pc.tile([128, RCMAX * heads], f32, tag="lr")
                    nc.vector.tensor_scalar_mul(
                        lr[:, :rc * heads], e_t[:, :rc * heads], NEG_SLOPE)
                    nc.vector.tensor_tensor(
                        out=lr[:, :rc * heads], in0=lr[:, :rc * heads],
                        in1=e_t[:, :rc * heads], op=OP.max)
                    nc.scalar.activation(
                        w_t[:, c0 * heads:(c0 + rc) * heads],
                        lr[:, :rc * heads], AF.Exp)
                    if not last:
                        nc.vector.tensor_tensor(
                            out=rhs[:, c0 * 72:(c0 + rc) * 72].rearrange(
                                "p (k x h) -> p k x h", x=9, h=8),
                            in0=w_t[:, c0 * heads:(c0 + rc) * heads]
                            .rearrange("p (k h) -> p k h", h=heads)
                            .unsqueeze(2).to_broadcast([128, rc, 9, 8]),
                            in1=g1r[:, c0:c0 + rc, 0:72].rearrange(
                                "p k (x h) -> p k x h", h=8),
                            op=OP.mult)

                if dbg is not None and l == 0 and g == 0:
                    nc.sync.dma_start(dbg["d_g1"], g1[:])
                    nc.sync.dma_start(dbg["d_st"], st[:])
                    nc.sync.dma_start(dbg["d_w"], w_t[:])

                for bloc in range(GSB):
                    bb = g * GSB + bloc
                    chs = meta.block_chunks[bb]
                    if last:
                        # ---- layer 2: per-head numerators + classifier ----
                        # wg8[p, j, head, 64] = w * G  built per shard-run
                        TBMX = meta.TBMAX
                        wg8 = pc.tile([128, TBMX * heads * hid], bf16,
                                      tag="wg8")
                        runs = meta.block_runs[bb]  # [(gsb-chunk, nchunks)]
                        loc = 0
                        for (ck0, nn) in runs:
                            nc.vector.tensor_tensor(
                                out=wg8[:, loc * heads * hid:
                                        (loc + nn) * heads * hid].rearrange(
                                    "p (k h c) -> p k h c", h=heads, c=hid),
                                in0=w_t[:, ck0 * heads:(ck0 + nn) * heads]
                                .rearrange("p (k h) -> p k h", h=heads)
                                .unsqueeze(3)
                                .to_broadcast([128, nn, heads, hid]),
                                in1=g1r[:, ck0:ck0 + nn, 0:hid]
                                .unsqueeze(2)
                                .to_broadcast([128, nn, heads, hid]),
                                op=OP.mult)
                            loc += nn
                        ps_num = pps.tile([128, heads * hid], f32, tag="num")
                        ps_den = pps1.tile([128, heads], f32, tag="den")
                        for ci, ck in enumerate(chs):
                            lhsT = st[:, ck * 128:(ck + 1) * 128]
                            nc.tensor.matmul(
                                ps_num[:], lhsT,
                                wg8[:, ci * heads * hid:
                                    (ci + 1) * heads * hid],
                                start=(ci == 0), stop=(ci == len(chs) - 1))
                            nc.tensor.matmul(
                                ps_den[:], lhsT,
                                w_t[:, ck * heads:(ck + 1) * heads],
                                start=(ci == 0), stop=(ci == len(chs) - 1))
                        # ---- implicit self-loop: local contribution ----
                        rt = pc.tile([128, ROWW], bf16, tag="rt")
                        nc.sync.dma_start(
                            rt[:], hxp[l][bb * 128:(bb + 1) * 128, :])
                        es = pc.tile([128, heads], f32, tag="es")
                        nc.vector.tensor_tensor(
                            out=es[:], in0=rt[:, hid + 8:hid + 16],
                            in1=stash[l % 2][:, bb * heads:(bb + 1) * heads],
                            op=OP.add)
                        ls = pc.tile([128, heads], f32, tag="ls")
                        nc.vector.tensor_scalar_mul(ls[:], es[:], NEG_SLOPE)
                        nc.vector.tensor_tensor(
                            out=ls[:], in0=ls[:], in1=es[:], op=OP.max)
                        ws = pc.tile([128, heads], f32, tag="ws")
                        nc.scalar.activation(ws[:], ls[:], AF.Exp)
                        prod8 = pc.tile([128, heads * hid], f32, tag="prod8")
                        nc.vector.tensor_tensor(
                            out=prod8[:].rearrange("p (h c) -> p h c", c=hid),
                            in0=rt[:, 0:hid].unsqueeze(1)
                            .to_broadcast([128, heads, hid]),
                            in1=ws[:].unsqueeze(2)
                            .to_broadcast([128, heads, hid]),
                            op=OP.mult)
                        if chs:
                            nc.vector.tensor_tensor(
                                out=ps_num[:], in0=ps_num[:], in1=prod8[:],
                                op=OP.add)
                            nc.vector.tensor_tensor(
                                out=ps_den[:], in0=ps_den[:], in1=ws[:],
                                op=OP.add)
                        else:
                            nc.vector.tensor_copy(ps_num[:], prod8[:])
                            nc.vector.tensor_copy(ps_den[:], ws[:])
                        recip = pc.tile([128, heads], f32, tag="recip2")
                        nc.vector.reciprocal(recip[:], ps_den[:])
                        fnum = pc.tile([128, heads * hid], f32, tag="fnum")
                        nc.vector.tensor_tensor(
                            out=fnum[:].rearrange("p (h c) -> p h c", c=hid),
                            in0=ps_num[:].rearrange("p (h c) -> p h c", c=hid),
                            in1=recip[:].unsqueeze(2)
                            .to_broadcast([128, heads, hid]),
                            op=OP.mult)
                        ps_o = pps1.tile([128, ncls], f32, tag="o")
                        for k in range(4):
                            ps_fT = pps.tile([128, 128], f32, tag="fT")
                            nc.tensor.transpose(
                                ps_fT[:], fnum[:, k * 128:(k + 1) * 128],
                                ident[:])
                            fT = pc.tile([128, 128], f32, tag="fTsb")
                            nc.vector.tensor_copy(fT[:], ps_fT[:])
                            nc.tensor.matmul(ps_o[:], fT[:], w2s[k][:],
                                             start=(k == 0), stop=(k == 3))
                        o_t = pc.tile([128, ncls], f32, tag="o_sb")
                        nc.vector.tensor_tensor(
                            out=o_t[:], in0=ps_o[:], in1=b2r[:], op=OP.add)
                        mx = pc.tile([128, 1], f32, tag="mx")
                        nc.vector.tensor_reduce(
                            mx[:], o_t[:], axis=mybir.AxisListType.X,
                            op=OP.max)
                        os_t = pc.tile([128, ncls], f32, tag="os")
                        nc.vector.tensor_tensor(
                            out=os_t[:], in0=o_t[:],
                            in1=mx[:].to_broadcast([128, ncls]),
                            op=OP.subtract)
                        ex = pc.tile([128, ncls], f32, tag="ex")
                        ssum = pc.tile([128, 1], f32, tag="ssum")
                        nc.scalar.activation(ex[:], os_t[:], AF.Exp,
                                             accum_out=ssum[:])
                        lse = pc.tile([128, 1], f32, tag="lse")
                        nc.scalar.activation(lse[:], ssum[:], AF.Ln)
                        res = pc.tile([128, ncls], f32, tag="res")
                        nc.vector.tensor_tensor(
                            out=res[:], in0=os_t[:],
                            in1=lse[:].to_broadcast([128, ncls]),
                            op=OP.subtract)
                        nc.sync.dma_start(
                            io["out"][bb * 128:(bb + 1) * 128, :], res[:])
                    else:
                        ps_agg = pps.tile([128, 72], f32, tag="agg")
                        for ci, ck in enumerate(chs):
                            nc.tensor.matmul(
                                ps_agg[:], st[:, ck * 128:(ck + 1) * 128],
                                rhs[:, ck * 72:(ck + 1) * 72],
                                start=(ci == 0), stop=(ci == len(chs) - 1))
                        # ---- implicit self-loop: local contribution ----
                        rt = pc.tile([128, ROWW], bf16, tag="rt")
                        nc.sync.dma_start(
                            rt[:], hxp[l][bb * 128:(bb + 1) * 128, :])
                        es = pc.tile([128, heads], f32, tag="es")
                        nc.vector.tensor_tensor(
                            out=es[:], in0=rt[:, hid + 8:hid + 16],
                            in1=stash[l % 2][:, bb * heads:(bb + 1) * heads],
                            op=OP.add)
                        ls = pc.tile([128, heads], f32, tag="ls")
                        nc.vector.tensor_scalar_mul(ls[:], es[:], NEG_SLOPE)
                        nc.vector.tensor_tensor(
                            out=ls[:], in0=ls[:], in1=es[:], op=OP.max)
                        ws = pc.tile([128, heads], f32, tag="ws")
                        nc.scalar.activation(ws[:], ls[:], AF.Exp)
                        prod = pc.tile([128, hid], f32, tag="prod")
                        nc.vector.tensor_tensor(
                            out=prod[:].rearrange("p (c h) -> p c h", h=heads),
                            in0=rt[:, 0:hid].rearrange(
                                "p (c h) -> p c h", h=heads),
                            in1=ws[:].unsqueeze(1)
                            .to_broadcast([128, C_HID, heads]),
                            op=OP.mult)
                        if chs:
                            nc.vector.tensor_tensor(
                                out=ps_agg[:, 0:hid], in0=ps_agg[:, 0:hid],
                                in1=prod[:], op=OP.add)
                            nc.vector.tensor_tensor(
                                out=ps_agg[:, hid:hid + heads],
                                in0=ps_agg[:, hid:hid + heads],
                                in1=ws[:], op=OP.add)
                        else:
                            nc.vector.tensor_copy(ps_agg[:, 0:hid], prod[:])
                            nc.vector.tensor_copy(
                                ps_agg[:, hid:hid + heads], ws[:])
                        # ---- finalize: num/den, bias, ELU ----
                        recip = pc.tile([128, heads], f32, tag="recip")
                        nc.vector.reciprocal(recip[:], ps_agg[:, hid:hid + 8])
                        f_t = pc.tile([128, hid], f32, tag="f")
                        nc.vector.tensor_tensor(
                            out=f_t[:].rearrange("p (c h) -> p c h", h=heads),
                            in0=ps_agg[:, 0:hid].rearrange(
                                "p (c h) -> p c h", h=heads),
                            in1=recip[:].unsqueeze(1)
                            .to_broadcast([128, C_HID, heads]),
                            op=OP.mult)
                        nc.vector.tensor_tensor(
                            out=f_t[:], in0=f_t[:], in1=brep[:], op=OP.add)
                        xneg = pc.tile([128, hid], f32, tag="xneg")
                        nc.vector.tensor_scalar_min(xneg[:], f_t[:], 0.0)
                        expn = pc.tile([128, hid], f32, tag="expn")
                        nc.scalar.activation(expn[:], xneg[:], AF.Exp)
                        xpos = pc.tile([128, hid], f32, tag="xpos")
                        nc.vector.tensor_scalar_max(xpos[:], f_t[:], 0.0)
                        fe = pc.tile([128, hid], f32, tag="fe")
                        nc.vector.tensor_tensor(
                            out=fe[:], in0=expn[:], in1=xpos[:], op=OP.add)
                        nc.vector.tensor_scalar_add(fe[:], fe[:], -1.0)
                        ps_ft = pps.tile([hid, 128], f32, tag="ft")
                        nc.tensor.transpose(ps_ft[:], fe[:], ident[:])
                        ft = pc.tile([hid, 128], f32, tag="ftsb")
                        nc.vector.tensor_copy(ft[:], ps_ft[:])
                        ps_hxn = pps.tile([128, 80], f32, tag="hxn")
                        nc.tensor.matmul(ps_hxn[:], ft[:], wnext[:],
                                         start=True, stop=True)
                        store_hx(pc, ps_hxn, l + 1, bb)
    return


# ----------------------------------------------------------------------------
# Program build + run
# ----------------------------------------------------------------------------

_NC_CACHE = {}


def _build_program(cfg: Cfg, meta: Meta, debug=False):
    from contextlib import ExitStack
    import concourse.tile as tile
    import concourse.mybir as mybir
    from concourse import bacc

    key = (cfg, meta.key(), debug)
    if key in _NC_CACHE:
        return _NC_CACHE[key]

    f32 = mybir.dt.float32
    f16 = mybir.dt.float16
    bf16 = mybir.dt.bfloat16
    i16 = mybir.dt.int16
    nc = bacc.Bacc("TRN2", target_bir_lowering=False, debug=False,
                   num_devices=cfg.m)

    io = {}

    def inp(name, shape, dtype):
        io[name] = nc.dram_tensor(name, list(shape), dtype,
                                  kind="ExternalInput").ap()

    inp("xT", (cfg.f_in, cfg.npad), bf16)
    inp("g1i", (cfg.ngsb, 128, meta.KMAX * 8), i16)
    inp("dstc", (cfg.ngsb, 128, meta.KMAX), f16)
    inp("dstcT", (cfg.ngsb, 1, meta.KMAX * 128), f16)
    inp("W0ext", (cfg.f_in, 80), bf16)
    inp("W1ext", (cfg.hid, 80), f32)
    inp("W2ext", (cfg.hid, 80), f32)
    inp("W2stack", (cfg.heads * cfg.hid, cfg.ncls), f32)
    inp("b0r", (128, cfg.hid), f32)
    inp("b1r", (128, cfg.hid), f32)
    inp("b2r", (128, cfg.ncls), f32)
    inp("iota", (128, 128), f16)
    inp("iota_col", (128, 1), f16)
    io["out"] = nc.dram_tensor("out", [cfg.npad, cfg.ncls], f32,
                               kind="ExternalOutput").ap()

    if debug:
        dbg_specs = {
            "d_g1": ([128, meta.KMAX * ROWW], bf16),
            "d_st": ([128, meta.KMAX * 128], bf16),
            "d_w": ([128, meta.KMAX * 8], bf16),
        }
        io["dbg"] = {
            name: nc.dram_tensor(name, list(shape), dt_,
                                 kind="ExternalOutput").ap()
            for name, (shape, dt_) in dbg_specs.items()
        }

    with tile.TileContext(nc) as tc:
        with ExitStack() as ctx:
            _emit_kernel(ctx, tc, cfg, meta, io)
    nc.compile()
    _NC_CACHE[key] = nc
    return nc


class _SimResults:
    def __init__(self, results):
        self.results = results
        self.exec_time_ns = None


def run(cfg: Cfg, inputs, trace=False, debug=False, sim=False):
    in_maps, meta = _preprocess(cfg, **inputs)
    nc = _build_program(cfg, meta, debug=debug)
    if sim:
        from concourse.bass_interp import MultiCoreSim
        ms = MultiCoreSim(nc, num_cores=cfg.m, trace=False,
                          require_finite=False, require_nnan=False)
        for mm in range(cfg.m):
            for k, v in in_maps[mm].items():
                ms.cores[mm].tensor(k)[:] = v
        ms.simulate(check_with_hw=False)
        results = []
        for mm in range(cfg.m):
            r = {"out": np.array(ms.cores[mm].mem_tensor("out"))}
            if debug:
                for nme in ("d_g1", "d_st", "d_w"):
                    r[nme] = np.array(ms.cores[mm].mem_tensor(nme))
            results.append(r)
        res = _SimResults(results)
    else:
        from concourse.bass_utils import run_bass_kernel_spmd
        res = run_bass_kernel_spmd(nc, in_maps, list(range(cfg.m)),
                                   trace=trace)
    out = np.concatenate(
        [res.results[mm]["out"][:cfg.npc] for mm in range(cfg.m)], axis=0)
    return out.astype(np.float32), res, cfg


def make_cfg(n=N, f_in=F_IN, ncls=N_CLS, m=M_CORES, gsb=None):
    npc = n // m
    b = int(math.ceil(npc / 128))
    if gsb is None:
        gsb = 2 if b % 2 == 0 else 1
    assert b % gsb == 0, (b, gsb)
    return Cfg(m=m, n=n, f_in=f_in, ncls=ncls, npc=npc, b=b, gsb=gsb)


def kernel(**inputs) -> np.ndarray:
    cfg = make_cfg()
    out, _, _ = run(cfg, inputs)
    return out
